# revision 27
# baseline (speedup 1.0000x reference)
"""Trainium2 Bass kernel for nn_CMFA (dense_transformer, seq_len=1 cross-attention).

Math notes (exact simplifications vs the reference):
  - softmax over a single key is exactly 1.0, so the attention output is
    exactly the v-projection: mha(q,k,v) = (v @ Wv.T + bv) @ Wo.T + bo.
    The q/k projections never influence the output.
  - Wv -> Wo -> fi2 is a linear chain (no nonlinearity), so it is folded on
    the host:  V = [v1, i_] @ Wcat.T + bcat  with
      Wcat = [fi2 @ (Wo @ Wv), fi2],  bcat = fi2 @ (Wo @ bv + bo) + fi2_b
    (the i_ column block carries the residual through fi2).

Precision: matmul operands (inputs, weights, intermediate activations) are
fp16; PSUM accumulation and biases are f32; output stores are f16 (host
upcasts).  Measured end-to-end error is 5.2e-4 (gate 2e-2).  fp8/DoubleRow
was evaluated and rejected: e4m3 on even one layer measures 3.7e-2 on the
harness metric (exact-input host simulation, calibrated to HW), and any
hi/lo residual correction costs at least as many PE cycles as fp16.

Device layout: activations are feature-major ("transposed", [feat, batch]) so
every matmul contracts over the partition dim and every DMA is contiguous.
The host pre-transposes the batch shards of i/t and transposes the output
back. Pure data parallel across 8 cores; weights replicated.

The kernel is PE-bound: 1312 matmuls x (512/2.4GHz + ~4ns NX) ~= 285us is
the fp16 floor, and the measured stream runs gap-free at that rate.  The
remaining schedule work is all at the edges (trace-measured):
  - The framework preamble (engine barriers + TENSOR_LOADs) runs to ~7us;
    nothing can issue before it.  First-use DMA-ring latency is absorbed by
    a 512B warmer on each HWDGE ring ahead of the gating loads.
  - The two loads that gate the first real layer (wt1 and x0's t-chunk) go
    one per HWDGE ring (sync/scalar) so their startup transfers overlap; a
    single N=128 warm-up matmul on wt1 covers the residual latency and
    opens the HAM busy window (PE runs at 1.2GHz until ~3.4us of sustained
    activity).
  - 8 cores share HBM, so the startup flood is rationed: the preamble
    issues ONLY tile-0's t_/i_ needs (x chunks paired with fi1 weight
    chunks, single-chunk groups first); wc/wV/wT/x-tile-1 issue from the
    scalar queue BEHIND act instructions, which gates their transfer on
    pipeline progress.  Later x tiles prefetch just-in-time via the
    bufs=2 x-pool WAR dependency.
  - Per tile: t_ first (tiny gate), then fi1 k-outer (4 matmuls per
    arriving chunk), then ct1/ci1, then the PREVIOUS tile's folded output
    layers (one-tile software pipeline keeps wV/wT deadlines late).
  - Output stores are f16 on the sync queue; the kernel's closing block
    computes in two half-column PSUM groups with acts split over
    scalar+vector and stores over both HWDGE rings, so the final
    store chain gates on 256 columns instead of 512.
"""

import numpy as np

B, IMG, TAB, HID = 32768, 2048, 128, 512
NCORES = 8
BS = B // NCORES  # rows per core
NT = 512          # batch-tile (matmul moving/free dim)
KI = IMG // 128   # 16 contraction chunks for fi1
XALL = KI + 1     # + the t chunk, packed as chunk 0 of the same tile
NWARM = 1         # PE p-state warm-up matmuls (on the wt1 tile, earliest load)

_CACHE = {}


def _pack_blocks(WT: np.ndarray, K: int, M: int) -> np.ndarray:
    """[K*128, M*128] -> [128, K, M*128] with [p, k, m*128+j] = WT[k*128+p, m*128+j]."""
    out = WT.reshape(K, 128, M * 128).transpose(1, 0, 2)
    return np.ascontiguousarray(out, dtype=np.float16)


def _build_nc(bs: int):
    import concourse.bass as bass
    import concourse.tile as tile
    from concourse import bacc, mybir

    f32 = mybir.dt.float32
    f16 = mybir.dt.float16
    Relu = mybir.ActivationFunctionType.Relu
    Ident = mybir.ActivationFunctionType.Identity
    ntiles = bs // NT

    nc = bacc.Bacc("TRN2", target_bir_lowering=False, debug=False)

    # tile-major input layout: per batch-tile, all 17 chunks contiguous per
    # partition (16KB lines -> large DMA descriptors, one prefetch per tile)
    iT_d = nc.dram_tensor("iT", [bs // NT, 128, XALL, NT], f16,
                          kind="ExternalInput").ap()
    w_fi1_d = nc.dram_tensor("w_fi1", [128, KI, 512], f16, kind="ExternalInput").ap()
    w_ft1_d = nc.dram_tensor("w_ft1", [128, 1, 512], f16, kind="ExternalInput").ap()
    w_ci1_d = nc.dram_tensor("w_ci1", [128, 4, 512], f16, kind="ExternalInput").ap()
    w_ct1_d = nc.dram_tensor("w_ct1", [128, 4, 512], f16, kind="ExternalInput").ap()
    w_V_d = nc.dram_tensor("w_V", [128, 8, 512], f16, kind="ExternalInput").ap()
    w_T_d = nc.dram_tensor("w_T", [128, 8, 512], f16, kind="ExternalInput").ap()
    bias_d = nc.dram_tensor("bias", [128, 24], f32, kind="ExternalInput").ap()
    # f16 output: halves store traffic and the closing transfer; adds only
    # ~1e-4 to the (2e-2-gated) rel err -- host upcasts to f32.
    out_d = nc.dram_tensor("outT", [2 * HID, bs], f16, kind="ExternalOutput").ap()

    # fi1 chunk groups for tile 0 (i-chunk indices): group completion is
    # all-or-nothing, so the FIRST groups are single chunks (earliest
    # availability while the PE is still ramping); the back groups are
    # coarse since they arrive far ahead of consumption anyway.  x tile
    # chunk index = i-chunk + 1 (chunk 0 is the t chunk).
    WGRP = [(0, 1), (1, 2), (2, 4), (4, 6), (6, 9), (9, 12), (12, 16)]

    with tile.TileContext(nc) as tc:
        with (
            tc.tile_pool(name="w", bufs=1) as wpool,
            tc.tile_pool(name="x", bufs=2) as xpool,
            tc.tile_pool(name="h", bufs=8) as hpool,
            tc.tile_pool(name="o", bufs=8) as opool,
            tc.tile_pool(name="ps", bufs=8, space="PSUM") as pspool,
        ):
            wf1 = wpool.tile([128, KI, 512], f16, name="w_fi1_t")
            wt1 = wpool.tile([128, 1, 512], f16, name="w_ft1_t")
            wc1 = wpool.tile([128, 4, 512], f16, name="w_ci1_t")
            wc2 = wpool.tile([128, 4, 512], f16, name="w_ct1_t")
            wV = wpool.tile([128, 8, 512], f16, name="w_V_t")
            wT = wpool.tile([128, 8, 512], f16, name="w_T_t")
            bt = wpool.tile([128, 24], f32, name="bias_t")

            # ---- preamble loads: ONLY what tile 0's t_/i_ layers need.
            # 8 cores flood the shared HBM at startup, so every byte issued
            # here delays the startup-critical chunks on every core.  All
            # later-needed tensors (wc, wV/wT, x tile 1) are issued from the
            # scalar queue BEHIND act instructions inside the n=0 body: the
            # scalar queue is FIFO and acts wait on PE sems, so those
            # transfers only start once tile 0 is well underway.
            # wt1 and x0's t-chunk gate the first real layer (t_): one on
            # each HWDGE ring so their ~2.7us startup transfers overlap.
            x_cur = xpool.tile([128, XALL, NT], f16, tag="x", name="x_0")
            x_1 = xpool.tile([128, XALL, NT], f16, tag="x", name="x_1")
            # ring warmers: a 512B transfer on each HWDGE ring absorbs the
            # first-use ring/doorbell latency ahead of the gating loads
            dwarm = wpool.tile([128, 2], f32, name="dma_warm")
            nc.sync.dma_start(dwarm[:, 0:1], bias_d[:, 0:1])
            nc.scalar.dma_start(dwarm[:, 1:2], bias_d[:, 1:2])
            nc.sync.dma_start(wt1[:], w_ft1_d[:])
            nc.scalar.dma_start(x_cur[:, 0, :], iT_d[0, :, 0, :])
            for gi, (a, b) in enumerate(WGRP):
                nc.sync.dma_start(x_cur[:, a + 1:b + 1, :], iT_d[0, :, a + 1:b + 1, :])
                nc.scalar.dma_start(wf1[:, a:b, :], w_fi1_d[:, a:b, :])
                if gi == 0:
                    # bias is only needed by the t_ act (~12us); k0's weight
                    # group outranks it on the scalar ring
                    nc.scalar.dma_start(bt[:], bias_d[:])

            # ---- PE warm-up on wt1 (N=128: cheap): starts the HAM busy
            # window early and bridges the t-chunk/bias DMA latency ----
            wps = pspool.tile([128, NT], f32, tag="ps", name="warm_ps")
            for _ in range(NWARM):
                nc.tensor.matmul(wps[:, 0:128], wt1[:, 0, 0:128], wt1[:, 0, 0:128],
                                 start=True, stop=True)

            def act(ps, htag, n, m, bcol, func):
                h = hpool.tile([128, NT], f16, tag=htag, name=f"{htag}_{n}_{m}")
                nc.scalar.activation(h[:], ps[:], func, bias=bt[:, bcol + m:bcol + m + 1])
                return h

            def layer_k_outer(wt, xs, htag, n, bcol, K):
                """All 4 output blocks accumulate in parallel, k outer: 4
                matmuls per input chunk k (rate-matches chunked DMA arrival)."""
                ps = [pspool.tile([128, NT], f32, tag="ps", name=f"ps_{htag}_{n}_{m}")
                      for m in range(4)]
                for k in range(K):
                    for m in range(4):
                        nc.tensor.matmul(ps[m][:], wt[:, k, m * 128:(m + 1) * 128],
                                         xs[k], start=(k == 0), stop=(k == K - 1))
                return [act(ps[m], htag, n, m, bcol, Relu) for m in range(4)]

            def layer_m_outer(wt, xs, htag, n, bcol, K):
                """m outer: each PSUM bank closes after its k loop and drains
                on the scalar engine while the PE works on the next block."""
                outs = []
                for m in range(4):
                    ps = pspool.tile([128, NT], f32, tag="ps", name=f"ps_{htag}_{n}_{m}")
                    for k in range(K):
                        nc.tensor.matmul(ps[:], wt[:, k, m * 128:(m + 1) * 128],
                                         xs[k], start=(k == 0), stop=(k == K - 1))
                    outs.append(act(ps, htag, n, m, bcol, Relu))
                return outs

            def cat_layer(wt, ts_a, ts_b, n, bcol, oname, orow0, final=False):
                """out[m] = sum_k w[k].T@xs_a[k] + w[4+k].T@xs_b[k] + bias; f16 store.

                The bias-add alternates between the scalar and (otherwise idle)
                vector engines so the final tile's four output blocks drain in
                parallel; stores issue from the sync queue, idle by then. The
                very last store (final, m=3) goes in two column halves on two
                queues so the closing transfer is half as long.
                """
                xs_a = [t[:] for t in ts_a]
                xs_b = [t[:] for t in ts_b]
                for m in range(4):
                    o = opool.tile([128, NT], f16, tag="o", name=f"o{oname}_{n}_{m}")
                    bias_ap = bt[:, bcol + m:bcol + m + 1]
                    rows = out_d[orow0 + 128 * m:orow0 + 128 * (m + 1),
                                 n * NT:(n + 1) * NT]
                    if final and m == 3:
                        # kernel's closing chain: two half-column PSUM groups,
                        # so half 0's act+store issue while half 1's matmuls
                        # still run, and the final act gates only 256 cols.
                        # Acts split over scalar+vector; stores over both
                        # HWDGE rings.
                        H = NT // 2
                        for h, (eng_act, eng_dma) in enumerate(
                                ((nc.scalar, nc.sync), (nc.vector, nc.scalar))):
                            ph = pspool.tile([128, H], f32, tag="ps",
                                             name=f"ps_{oname}_{n}_3{h}")
                            cs = slice(h * H, h * H + H)
                            for k in range(4):
                                nc.tensor.matmul(
                                    ph[:], wt[:, k, m * 128:(m + 1) * 128],
                                    ts_a[k][:, cs], start=(k == 0), stop=False)
                            for k in range(4):
                                nc.tensor.matmul(
                                    ph[:], wt[:, 4 + k, m * 128:(m + 1) * 128],
                                    ts_b[k][:, cs], start=False, stop=(k == 3))
                            if eng_act is nc.scalar:
                                nc.scalar.activation(o[:, cs], ph[:], Ident,
                                                     bias=bias_ap)
                            else:
                                nc.vector.tensor_scalar_add(o[:, cs], ph[:],
                                                            bias_ap)
                            eng_dma.dma_start(rows[:, cs], o[:, cs])
                        continue
                    ps = pspool.tile([128, NT], f32, tag="ps", name=f"ps_{oname}_{n}_{m}")
                    for k in range(4):
                        nc.tensor.matmul(ps[:], wt[:, k, m * 128:(m + 1) * 128],
                                         xs_a[k], start=(k == 0), stop=False)
                    for k in range(4):
                        nc.tensor.matmul(ps[:], wt[:, 4 + k, m * 128:(m + 1) * 128],
                                         xs_b[k], start=False, stop=(k == 3))
                    if m % 2 == 0:
                        nc.scalar.activation(o[:], ps[:], Ident, bias=bias_ap)
                    else:
                        nc.vector.tensor_scalar_add(o[:], ps[:], bias_ap)
                    nc.sync.dma_start(rows, o[:])

            def vt_phase(n, v1, i_, v2, t_, final=False):
                # ---- V = [v1, i_] @ WcatV.T + bcatV ----
                cat_layer(wV, v1, i_, n, 16, "V", 0)
                # ---- T = [v2, t_] @ WcatT.T + bcatT ----
                cat_layer(wT, v2, t_, n, 20, "T", HID, final=final)

            xtiles = [x_cur, x_1]
            prev = None
            for n in range(ntiles):
                x_n = xtiles[n]
                xs_i = [x_n[:, k + 1, :] for k in range(KI)]

                # JIT prefetch of tile n+1 (x pool bufs=2: the issue WARs the
                # tile n-1 slot, so the transfer starts right as tile n does
                # -- a full tile-time (~35us) ahead of need, and never during
                # the 8-core startup HBM crunch)
                if n >= 1 and n + 1 < ntiles:
                    x_nxt = xpool.tile([128, XALL, NT], f16, tag="x", name=f"x_{n + 1}")
                    nc.sync.dma_start(x_nxt[:, 0:9, :], iT_d[n + 1, :, 0:9, :])
                    nc.sync.dma_start(x_nxt[:, 9:XALL, :], iT_d[n + 1, :, 9:XALL, :])
                    xtiles.append(x_nxt)

                # ---- t_ = relu(t @ ft1.T + b): gates on only 256KB of input ----
                t_ = layer_m_outer(wt1, [x_n[:, 0, :]], "t_", n, 4, 1)
                if n == 0:
                    # wc loads ride the scalar queue behind the t_ acts:
                    # transfer starts once tile 0 is underway, lands well
                    # before the v2/v1 layers need them
                    nc.scalar.dma_start(wc2[:], w_ct1_d[:])
                    nc.scalar.dma_start(wc1[:], w_ci1_d[:])
                # ---- i_ = relu(i @ fi1.T + b) ----
                i_ = layer_k_outer(wf1, xs_i, "i_", n, 0, KI)
                if n == 0:
                    # x tile 1 behind the i_ acts (needed from ~tile 1 mid)
                    nc.scalar.dma_start(x_1[:, 0:9, :], iT_d[1, :, 0:9, :])
                    nc.scalar.dma_start(x_1[:, 9:XALL, :], iT_d[1, :, 9:XALL, :])

                # ---- v2 = relu(t_ @ ct1.T + b): fills the i_ activation latency ----
                v2 = layer_m_outer(wc2, [h[:] for h in t_], "v2", n, 12, 4)
                # ---- v1 = relu(i_ @ ci1.T + b) ----
                v1 = layer_m_outer(wc1, [h[:] for h in i_], "v1", n, 8, 4)
                if n == 0:
                    # cat weights behind the v2/v1 acts (needed from the
                    # deferred vt_phase(0), which runs inside tile 1)
                    nc.scalar.dma_start(wV[:, 0:4, :], w_V_d[:, 0:4, :])
                    nc.scalar.dma_start(wV[:, 4:8, :], w_V_d[:, 4:8, :])
                    nc.scalar.dma_start(wT[:, 0:4, :], w_T_d[:, 0:4, :])
                    nc.scalar.dma_start(wT[:, 4:8, :], w_T_d[:, 4:8, :])

                # ---- V/T of the PREVIOUS tile (one-tile software pipeline:
                # moves the wV/wT load deadlines out of the startup crunch) ----
                if prev is not None:
                    vt_phase(n - 1, *prev)
                prev = (v1, i_, v2, t_)

            vt_phase(ntiles - 1, *prev, final=True)

    nc.compile()
    return nc


def _host_pack(inp: dict):
    f8 = np.float64
    fi1_w, fi1_b = inp["fi1_w"], inp["fi1_b"]
    ft1_w, ft1_b = inp["ft1_w"], inp["ft1_b"]
    ci1_w, ci1_b = inp["ci1_w"], inp["ci1_b"]
    ct1_w, ct1_b = inp["ct1_w"], inp["ct1_b"]

    def fold(wv, bv, wo, bo, f_w, f_b):
        Wvo = wo.astype(f8) @ wv.astype(f8)
        bvo = wo.astype(f8) @ bv.astype(f8) + bo.astype(f8)
        Wcat = np.concatenate([f_w.astype(f8) @ Wvo, f_w.astype(f8)], axis=1)
        bcat = f_w.astype(f8) @ bvo + f_b.astype(f8)
        return Wcat.astype(np.float32), bcat.astype(np.float32)

    WcatV, bcatV = fold(inp["aV_wv"], inp["aV_bv"], inp["aV_wo"], inp["aV_bo"],
                        inp["fi2_w"], inp["fi2_b"])
    WcatT, bcatT = fold(inp["aT_wv"], inp["aT_bv"], inp["aT_wo"], inp["aT_bo"],
                        inp["ft2_w"], inp["ft2_b"])

    weights = {
        "w_fi1": _pack_blocks(np.ascontiguousarray(fi1_w.T), 16, 4),
        "w_ft1": _pack_blocks(np.ascontiguousarray(ft1_w.T), 1, 4),
        "w_ci1": _pack_blocks(np.ascontiguousarray(ci1_w.T), 4, 4),
        "w_ct1": _pack_blocks(np.ascontiguousarray(ct1_w.T), 4, 4),
        "w_V": _pack_blocks(np.ascontiguousarray(WcatV.T), 8, 4),
        "w_T": _pack_blocks(np.ascontiguousarray(WcatT.T), 8, 4),
    }
    cols = []
    for b in (fi1_b, ft1_b, ci1_b, ct1_b, bcatV, bcatT):
        for m in range(4):
            cols.append(b[128 * m:128 * (m + 1)])
    weights["bias"] = np.ascontiguousarray(np.stack(cols, axis=1), dtype=np.float32)
    return weights


def make_in_maps(inputs: dict):
    """Full inputs -> per-core input dicts (shard batch, replicate weights)."""
    inputs = {k: np.asarray(v) for k, v in inputs.items()}
    i = np.asarray(inputs["i"], dtype=np.float32)
    t = np.asarray(inputs["t"], dtype=np.float32)
    weights = _host_pack(inputs)
    i16 = i.astype(np.float16)
    t16 = t.astype(np.float16)
    ntiles = BS // NT
    in_maps = []
    for c in range(NCORES):
        sl = slice(c * BS, (c + 1) * BS)
        m = dict(weights)
        # [ntiles, 128, XALL, NT]: batch-tile major; chunk 0 holds
        # t[n*NT+j, p], chunk 1+k holds i[n*NT+j, 128k+p].
        xi = i16[sl].T.reshape(KI, 128, ntiles, NT)   # [k, p, n, j]
        xt = t16[sl].T.reshape(TAB, ntiles, NT)       # [p, n, j]
        full = np.empty((ntiles, 128, XALL, NT), dtype=np.float16)
        full[:, :, 0, :] = xt.transpose(1, 0, 2)
        full[:, :, 1:, :] = xi.transpose(2, 1, 0, 3)
        m["iT"] = full
        in_maps.append(m)
    return in_maps


def kernel(**inputs) -> np.ndarray:
    from concourse import bass_utils

    if "nc" not in _CACHE:
        _CACHE["nc"] = _build_nc(BS)
    nc = _CACHE["nc"]

    in_maps = make_in_maps(inputs)
    res = bass_utils.run_bass_kernel_spmd(nc, in_maps, core_ids=list(range(NCORES)))

    out = np.empty((B, 2 * HID), dtype=np.float32)
    for c in range(NCORES):
        out[c * BS:(c + 1) * BS] = res.results[c]["outT"].T.astype(np.float32)
    return out



# revision 29
# speedup vs baseline: 1.0064x; 1.0064x over previous
"""Trainium2 Bass kernel for nn_CMFA (dense_transformer, seq_len=1 cross-attention).

Math notes (exact simplifications vs the reference):
  - softmax over a single key is exactly 1.0, so the attention output is
    exactly the v-projection: mha(q,k,v) = (v @ Wv.T + bv) @ Wo.T + bo.
    The q/k projections never influence the output.
  - Wv -> Wo -> fi2 is a linear chain (no nonlinearity), so it is folded on
    the host:  V = [v1, i_] @ Wcat.T + bcat  with
      Wcat = [fi2 @ (Wo @ Wv), fi2],  bcat = fi2 @ (Wo @ bv + bo) + fi2_b
    (the i_ column block carries the residual through fi2).

Precision: matmul operands (inputs, weights, intermediate activations) are
fp16; PSUM accumulation and biases are f32; output stores are f16 (host
upcasts).  Measured end-to-end error is 5.2e-4 (gate 2e-2).  fp8/DoubleRow
was evaluated and rejected: e4m3 on even one layer measures 3.7e-2 on the
harness metric (exact-input host simulation, calibrated to HW), and any
hi/lo residual correction costs at least as many PE cycles as fp16.

Device layout: activations are feature-major ("transposed", [feat, batch]) so
every matmul contracts over the partition dim and every DMA is contiguous.
The host pre-transposes the batch shards of i/t and transposes the output
back. Pure data parallel across 8 cores; weights replicated.

The kernel is PE-bound: 1312 matmuls x (512/2.4GHz + ~4ns NX) ~= 285us is
the fp16 floor, and the measured stream runs gap-free at that rate.  The
remaining schedule work is all at the edges (trace-measured):
  - The framework preamble (engine barriers + TENSOR_LOADs) runs to ~7us;
    nothing can issue before it.  (A 512B first-DMA "ring warmer" per HWDGE
    ring was tried and is a net loss: it spends a ~0.7us FIFO issue slot
    ahead of the gating loads.)
  - The two loads that gate the first real layer (wt1 and x0's t-chunk) go
    one per HWDGE ring (sync/scalar) so their startup transfers overlap; a
    single N=128 warm-up matmul on wt1 covers the residual latency and
    opens the HAM busy window (PE runs at 1.2GHz until ~3.4us of sustained
    activity).
  - 8 cores share HBM, so the startup flood is rationed: the preamble
    issues ONLY tile-0's t_/i_ needs (x chunks paired with fi1 weight
    chunks, single-chunk groups first); wc/wV/wT/x-tile-1 issue from the
    scalar queue BEHIND act instructions, which gates their transfer on
    pipeline progress.  Later x tiles prefetch just-in-time via the
    bufs=2 x-pool WAR dependency.
  - Per tile: t_ first (tiny gate), then fi1 k-outer (4 matmuls per
    arriving chunk), then ct1/ci1, then the PREVIOUS tile's folded output
    layers (one-tile software pipeline keeps wV/wT deadlines late).
  - Output stores are f16 on the sync queue; the kernel's closing block
    computes in two half-column PSUM groups with acts split over
    scalar+vector and stores over both HWDGE rings, so the final
    store chain gates on 256 columns instead of 512.
"""

import numpy as np

B, IMG, TAB, HID = 32768, 2048, 128, 512
NCORES = 8
BS = B // NCORES  # rows per core
NT = 512          # batch-tile (matmul moving/free dim)
KI = IMG // 128   # 16 contraction chunks for fi1
XALL = KI + 1     # + the t chunk, packed as chunk 0 of the same tile
NWARM = 1         # PE p-state warm-up matmuls (on the wt1 tile, earliest load)

_CACHE = {}


def _pack_blocks(WT: np.ndarray, K: int, M: int) -> np.ndarray:
    """[K*128, M*128] -> [128, K, M*128] with [p, k, m*128+j] = WT[k*128+p, m*128+j]."""
    out = WT.reshape(K, 128, M * 128).transpose(1, 0, 2)
    return np.ascontiguousarray(out, dtype=np.float16)


def _build_nc(bs: int):
    import concourse.bass as bass
    import concourse.tile as tile
    from concourse import bacc, mybir

    f32 = mybir.dt.float32
    f16 = mybir.dt.float16
    Relu = mybir.ActivationFunctionType.Relu
    Ident = mybir.ActivationFunctionType.Identity
    ntiles = bs // NT

    nc = bacc.Bacc("TRN2", target_bir_lowering=False, debug=False)

    # tile-major input layout: per batch-tile, all 17 chunks contiguous per
    # partition (16KB lines -> large DMA descriptors, one prefetch per tile)
    iT_d = nc.dram_tensor("iT", [bs // NT, 128, XALL, NT], f16,
                          kind="ExternalInput").ap()
    w_fi1_d = nc.dram_tensor("w_fi1", [128, KI, 512], f16, kind="ExternalInput").ap()
    w_ft1_d = nc.dram_tensor("w_ft1", [128, 1, 512], f16, kind="ExternalInput").ap()
    w_ci1_d = nc.dram_tensor("w_ci1", [128, 4, 512], f16, kind="ExternalInput").ap()
    w_ct1_d = nc.dram_tensor("w_ct1", [128, 4, 512], f16, kind="ExternalInput").ap()
    w_V_d = nc.dram_tensor("w_V", [128, 8, 512], f16, kind="ExternalInput").ap()
    w_T_d = nc.dram_tensor("w_T", [128, 8, 512], f16, kind="ExternalInput").ap()
    bias_d = nc.dram_tensor("bias", [128, 24], f32, kind="ExternalInput").ap()
    # f16 output: halves store traffic and the closing transfer; adds only
    # ~1e-4 to the (2e-2-gated) rel err -- host upcasts to f32.
    out_d = nc.dram_tensor("outT", [2 * HID, bs], f16, kind="ExternalOutput").ap()

    # fi1 chunk groups for tile 0 (i-chunk indices): group completion is
    # all-or-nothing, so the FIRST groups are single chunks (earliest
    # availability while the PE is still ramping); the back groups are
    # coarse since they arrive far ahead of consumption anyway.  x tile
    # chunk index = i-chunk + 1 (chunk 0 is the t chunk).
    WGRP = [(0, 1), (1, 2), (2, 4), (4, 6), (6, 9), (9, 12), (12, 16)]

    with tile.TileContext(nc) as tc:
        with (
            tc.tile_pool(name="w", bufs=1) as wpool,
            tc.tile_pool(name="x", bufs=2) as xpool,
            tc.tile_pool(name="h", bufs=8) as hpool,
            tc.tile_pool(name="o", bufs=8) as opool,
            tc.tile_pool(name="ps", bufs=8, space="PSUM") as pspool,
        ):
            wf1 = wpool.tile([128, KI, 512], f16, name="w_fi1_t")
            wt1 = wpool.tile([128, 1, 512], f16, name="w_ft1_t")
            wc1 = wpool.tile([128, 4, 512], f16, name="w_ci1_t")
            wc2 = wpool.tile([128, 4, 512], f16, name="w_ct1_t")
            wV = wpool.tile([128, 8, 512], f16, name="w_V_t")
            wT = wpool.tile([128, 8, 512], f16, name="w_T_t")
            bt = wpool.tile([128, 24], f32, name="bias_t")

            # ---- preamble loads: ONLY what tile 0's t_/i_ layers need.
            # 8 cores flood the shared HBM at startup, so every byte issued
            # here delays the startup-critical chunks on every core.  All
            # later-needed tensors (wc, wV/wT, x tile 1) are issued from the
            # scalar queue BEHIND act instructions inside the n=0 body: the
            # scalar queue is FIFO and acts wait on PE sems, so those
            # transfers only start once tile 0 is well underway.
            # wt1 and x0's t-chunk gate the first real layer (t_): one on
            # each HWDGE ring so their ~2.7us startup transfers overlap.
            x_cur = xpool.tile([128, XALL, NT], f16, tag="x", name="x_0")
            x_1 = xpool.tile([128, XALL, NT], f16, tag="x", name="x_1")
            nc.sync.dma_start(wt1[:], w_ft1_d[:])
            nc.scalar.dma_start(x_cur[:, 0, :], iT_d[0, :, 0, :])
            for gi, (a, b) in enumerate(WGRP):
                nc.sync.dma_start(x_cur[:, a + 1:b + 1, :], iT_d[0, :, a + 1:b + 1, :])
                nc.scalar.dma_start(wf1[:, a:b, :], w_fi1_d[:, a:b, :])
                if gi == 0:
                    # bias is only needed by the t_ act (~12us); k0's weight
                    # group outranks it on the scalar ring
                    nc.scalar.dma_start(bt[:], bias_d[:])

            # ---- PE warm-up on wt1 (N=128: cheap): starts the HAM busy
            # window early and bridges the t-chunk/bias DMA latency ----
            wps = pspool.tile([128, NT], f32, tag="ps", name="warm_ps")
            for _ in range(NWARM):
                nc.tensor.matmul(wps[:, 0:128], wt1[:, 0, 0:128], wt1[:, 0, 0:128],
                                 start=True, stop=True)

            def act(ps, htag, n, m, bcol, func):
                h = hpool.tile([128, NT], f16, tag=htag, name=f"{htag}_{n}_{m}")
                nc.scalar.activation(h[:], ps[:], func, bias=bt[:, bcol + m:bcol + m + 1])
                return h

            def layer_k_outer(wt, xs, htag, n, bcol, K):
                """All 4 output blocks accumulate in parallel, k outer: 4
                matmuls per input chunk k (rate-matches chunked DMA arrival)."""
                ps = [pspool.tile([128, NT], f32, tag="ps", name=f"ps_{htag}_{n}_{m}")
                      for m in range(4)]
                for k in range(K):
                    for m in range(4):
                        nc.tensor.matmul(ps[m][:], wt[:, k, m * 128:(m + 1) * 128],
                                         xs[k], start=(k == 0), stop=(k == K - 1))
                return [act(ps[m], htag, n, m, bcol, Relu) for m in range(4)]

            def layer_m_outer(wt, xs, htag, n, bcol, K):
                """m outer: each PSUM bank closes after its k loop and drains
                on the scalar engine while the PE works on the next block."""
                outs = []
                for m in range(4):
                    ps = pspool.tile([128, NT], f32, tag="ps", name=f"ps_{htag}_{n}_{m}")
                    for k in range(K):
                        nc.tensor.matmul(ps[:], wt[:, k, m * 128:(m + 1) * 128],
                                         xs[k], start=(k == 0), stop=(k == K - 1))
                    outs.append(act(ps, htag, n, m, bcol, Relu))
                return outs

            def cat_layer(wt, ts_a, ts_b, n, bcol, oname, orow0, final=False):
                """out[m] = sum_k w[k].T@xs_a[k] + w[4+k].T@xs_b[k] + bias; f16 store.

                The bias-add alternates between the scalar and (otherwise idle)
                vector engines so the final tile's four output blocks drain in
                parallel; stores issue from the sync queue, idle by then. The
                very last store (final, m=3) goes in two column halves on two
                queues so the closing transfer is half as long.
                """
                xs_a = [t[:] for t in ts_a]
                xs_b = [t[:] for t in ts_b]
                for m in range(4):
                    o = opool.tile([128, NT], f16, tag="o", name=f"o{oname}_{n}_{m}")
                    bias_ap = bt[:, bcol + m:bcol + m + 1]
                    rows = out_d[orow0 + 128 * m:orow0 + 128 * (m + 1),
                                 n * NT:(n + 1) * NT]
                    if final and m == 3:
                        # kernel's closing chain: two half-column PSUM groups,
                        # so half 0's act+store issue while half 1's matmuls
                        # still run, and the final act gates only 256 cols.
                        # Acts split over scalar+vector; stores over both
                        # HWDGE rings.
                        H = NT // 2
                        for h, (eng_act, eng_dma) in enumerate(
                                ((nc.scalar, nc.sync), (nc.vector, nc.scalar))):
                            ph = pspool.tile([128, H], f32, tag="ps",
                                             name=f"ps_{oname}_{n}_3{h}")
                            cs = slice(h * H, h * H + H)
                            for k in range(4):
                                nc.tensor.matmul(
                                    ph[:], wt[:, k, m * 128:(m + 1) * 128],
                                    ts_a[k][:, cs], start=(k == 0), stop=False)
                            for k in range(4):
                                nc.tensor.matmul(
                                    ph[:], wt[:, 4 + k, m * 128:(m + 1) * 128],
                                    ts_b[k][:, cs], start=False, stop=(k == 3))
                            if eng_act is nc.scalar:
                                nc.scalar.activation(o[:, cs], ph[:], Ident,
                                                     bias=bias_ap)
                            else:
                                nc.vector.tensor_scalar_add(o[:, cs], ph[:],
                                                            bias_ap)
                            eng_dma.dma_start(rows[:, cs], o[:, cs])
                        continue
                    ps = pspool.tile([128, NT], f32, tag="ps", name=f"ps_{oname}_{n}_{m}")
                    for k in range(4):
                        nc.tensor.matmul(ps[:], wt[:, k, m * 128:(m + 1) * 128],
                                         xs_a[k], start=(k == 0), stop=False)
                    for k in range(4):
                        nc.tensor.matmul(ps[:], wt[:, 4 + k, m * 128:(m + 1) * 128],
                                         xs_b[k], start=False, stop=(k == 3))
                    if m % 2 == 0:
                        nc.scalar.activation(o[:], ps[:], Ident, bias=bias_ap)
                    else:
                        nc.vector.tensor_scalar_add(o[:], ps[:], bias_ap)
                    nc.sync.dma_start(rows, o[:])

            def vt_phase(n, v1, i_, v2, t_, final=False):
                # ---- V = [v1, i_] @ WcatV.T + bcatV ----
                cat_layer(wV, v1, i_, n, 16, "V", 0)
                # ---- T = [v2, t_] @ WcatT.T + bcatT ----
                cat_layer(wT, v2, t_, n, 20, "T", HID, final=final)

            xtiles = [x_cur, x_1]
            prev = None
            for n in range(ntiles):
                x_n = xtiles[n]
                xs_i = [x_n[:, k + 1, :] for k in range(KI)]

                # JIT prefetch of tile n+1 (x pool bufs=2: the issue WARs the
                # tile n-1 slot, so the transfer starts right as tile n does
                # -- a full tile-time (~35us) ahead of need, and never during
                # the 8-core startup HBM crunch)
                if n >= 1 and n + 1 < ntiles:
                    x_nxt = xpool.tile([128, XALL, NT], f16, tag="x", name=f"x_{n + 1}")
                    nc.sync.dma_start(x_nxt[:, 0:9, :], iT_d[n + 1, :, 0:9, :])
                    nc.sync.dma_start(x_nxt[:, 9:XALL, :], iT_d[n + 1, :, 9:XALL, :])
                    xtiles.append(x_nxt)

                # ---- t_ = relu(t @ ft1.T + b): gates on only 256KB of input ----
                t_ = layer_m_outer(wt1, [x_n[:, 0, :]], "t_", n, 4, 1)
                if n == 0:
                    # wc loads ride the scalar queue behind the t_ acts:
                    # transfer starts once tile 0 is underway, lands well
                    # before the v2/v1 layers need them
                    nc.scalar.dma_start(wc2[:], w_ct1_d[:])
                    nc.scalar.dma_start(wc1[:], w_ci1_d[:])
                # ---- i_ = relu(i @ fi1.T + b) ----
                i_ = layer_k_outer(wf1, xs_i, "i_", n, 0, KI)
                if n == 0:
                    # x tile 1 behind the i_ acts (needed from ~tile 1 mid)
                    nc.scalar.dma_start(x_1[:, 0:9, :], iT_d[1, :, 0:9, :])
                    nc.scalar.dma_start(x_1[:, 9:XALL, :], iT_d[1, :, 9:XALL, :])

                # ---- v2 = relu(t_ @ ct1.T + b): fills the i_ activation latency ----
                v2 = layer_m_outer(wc2, [h[:] for h in t_], "v2", n, 12, 4)
                # ---- v1 = relu(i_ @ ci1.T + b) ----
                v1 = layer_m_outer(wc1, [h[:] for h in i_], "v1", n, 8, 4)
                if n == 0:
                    # cat weights behind the v2/v1 acts (needed from the
                    # deferred vt_phase(0), which runs inside tile 1)
                    nc.scalar.dma_start(wV[:, 0:4, :], w_V_d[:, 0:4, :])
                    nc.scalar.dma_start(wV[:, 4:8, :], w_V_d[:, 4:8, :])
                    nc.scalar.dma_start(wT[:, 0:4, :], w_T_d[:, 0:4, :])
                    nc.scalar.dma_start(wT[:, 4:8, :], w_T_d[:, 4:8, :])

                # ---- V/T of the PREVIOUS tile (one-tile software pipeline:
                # moves the wV/wT load deadlines out of the startup crunch) ----
                if prev is not None:
                    vt_phase(n - 1, *prev)
                prev = (v1, i_, v2, t_)

            vt_phase(ntiles - 1, *prev, final=True)

    nc.compile()
    return nc


def _host_pack(inp: dict):
    f8 = np.float64
    fi1_w, fi1_b = inp["fi1_w"], inp["fi1_b"]
    ft1_w, ft1_b = inp["ft1_w"], inp["ft1_b"]
    ci1_w, ci1_b = inp["ci1_w"], inp["ci1_b"]
    ct1_w, ct1_b = inp["ct1_w"], inp["ct1_b"]

    def fold(wv, bv, wo, bo, f_w, f_b):
        Wvo = wo.astype(f8) @ wv.astype(f8)
        bvo = wo.astype(f8) @ bv.astype(f8) + bo.astype(f8)
        Wcat = np.concatenate([f_w.astype(f8) @ Wvo, f_w.astype(f8)], axis=1)
        bcat = f_w.astype(f8) @ bvo + f_b.astype(f8)
        return Wcat.astype(np.float32), bcat.astype(np.float32)

    WcatV, bcatV = fold(inp["aV_wv"], inp["aV_bv"], inp["aV_wo"], inp["aV_bo"],
                        inp["fi2_w"], inp["fi2_b"])
    WcatT, bcatT = fold(inp["aT_wv"], inp["aT_bv"], inp["aT_wo"], inp["aT_bo"],
                        inp["ft2_w"], inp["ft2_b"])

    weights = {
        "w_fi1": _pack_blocks(np.ascontiguousarray(fi1_w.T), 16, 4),
        "w_ft1": _pack_blocks(np.ascontiguousarray(ft1_w.T), 1, 4),
        "w_ci1": _pack_blocks(np.ascontiguousarray(ci1_w.T), 4, 4),
        "w_ct1": _pack_blocks(np.ascontiguousarray(ct1_w.T), 4, 4),
        "w_V": _pack_blocks(np.ascontiguousarray(WcatV.T), 8, 4),
        "w_T": _pack_blocks(np.ascontiguousarray(WcatT.T), 8, 4),
    }
    cols = []
    for b in (fi1_b, ft1_b, ci1_b, ct1_b, bcatV, bcatT):
        for m in range(4):
            cols.append(b[128 * m:128 * (m + 1)])
    weights["bias"] = np.ascontiguousarray(np.stack(cols, axis=1), dtype=np.float32)
    return weights


def make_in_maps(inputs: dict):
    """Full inputs -> per-core input dicts (shard batch, replicate weights)."""
    inputs = {k: np.asarray(v) for k, v in inputs.items()}
    i = np.asarray(inputs["i"], dtype=np.float32)
    t = np.asarray(inputs["t"], dtype=np.float32)
    weights = _host_pack(inputs)
    i16 = i.astype(np.float16)
    t16 = t.astype(np.float16)
    ntiles = BS // NT
    in_maps = []
    for c in range(NCORES):
        sl = slice(c * BS, (c + 1) * BS)
        m = dict(weights)
        # [ntiles, 128, XALL, NT]: batch-tile major; chunk 0 holds
        # t[n*NT+j, p], chunk 1+k holds i[n*NT+j, 128k+p].
        xi = i16[sl].T.reshape(KI, 128, ntiles, NT)   # [k, p, n, j]
        xt = t16[sl].T.reshape(TAB, ntiles, NT)       # [p, n, j]
        full = np.empty((ntiles, 128, XALL, NT), dtype=np.float16)
        full[:, :, 0, :] = xt.transpose(1, 0, 2)
        full[:, :, 1:, :] = xi.transpose(2, 1, 0, 3)
        m["iT"] = full
        in_maps.append(m)
    return in_maps


def kernel(**inputs) -> np.ndarray:
    from concourse import bass_utils

    if "nc" not in _CACHE:
        _CACHE["nc"] = _build_nc(BS)
    nc = _CACHE["nc"]

    in_maps = make_in_maps(inputs)
    res = bass_utils.run_bass_kernel_spmd(nc, in_maps, core_ids=list(range(NCORES)))

    out = np.empty((B, 2 * HID), dtype=np.float32)
    for c in range(NCORES):
        out[c * BS:(c + 1) * BS] = res.results[c]["outT"].T.astype(np.float32)
    return out



# revision 31
# speedup vs baseline: 1.0104x; 1.0041x over previous
"""Trainium2 Bass kernel for nn_CMFA (dense_transformer, seq_len=1 cross-attention).

Math notes (exact simplifications vs the reference):
  - softmax over a single key is exactly 1.0, so the attention output is
    exactly the v-projection: mha(q,k,v) = (v @ Wv.T + bv) @ Wo.T + bo.
    The q/k projections never influence the output.
  - Wv -> Wo -> fi2 is a linear chain (no nonlinearity), so it is folded on
    the host:  V = [v1, i_] @ Wcat.T + bcat  with
      Wcat = [fi2 @ (Wo @ Wv), fi2],  bcat = fi2 @ (Wo @ bv + bo) + fi2_b
    (the i_ column block carries the residual through fi2).

Precision: matmul operands (inputs, weights, intermediate activations) are
fp16; PSUM accumulation and biases are f32; output stores are f16 (host
upcasts).  Measured end-to-end error is 5.2e-4 (gate 2e-2).  fp8/DoubleRow
was evaluated and rejected: e4m3 on even one layer measures 3.7e-2 on the
harness metric (exact-input host simulation, calibrated to HW), and any
hi/lo residual correction costs at least as many PE cycles as fp16.

Device layout: activations are feature-major ("transposed", [feat, batch]) so
every matmul contracts over the partition dim and every DMA is contiguous.
The host pre-transposes the batch shards of i/t and transposes the output
back. Pure data parallel across 8 cores; weights replicated.

The kernel is PE-bound: 1312 matmuls x (512/2.4GHz + ~4ns NX) ~= 285us is
the fp16 floor, and the measured stream runs gap-free at that rate.  The
remaining schedule work is all at the edges (trace-measured):
  - The framework preamble (engine barriers + TENSOR_LOADs) runs to ~7us;
    nothing can issue before it.  (A 512B first-DMA "ring warmer" per HWDGE
    ring was tried and is a net loss: it spends a ~0.7us FIFO issue slot
    ahead of the gating loads.)
  - The two loads that gate the first real layer (wt1 and x0's t-chunk) go
    one per HWDGE ring (sync/scalar) so their startup transfers overlap; a
    single N=128 warm-up matmul on wt1 covers the residual latency and
    opens the HAM busy window (PE runs at 1.2GHz until ~3.4us of sustained
    activity).
  - 8 cores share HBM, so the startup flood is rationed: the preamble
    issues ONLY tile-0's t_/i_ needs (x chunks paired with fi1 weight
    chunks, single-chunk groups first); wc/wV/wT/x-tile-1 issue from the
    scalar queue BEHIND act instructions, which gates their transfer on
    pipeline progress.  Later x tiles prefetch just-in-time via the
    bufs=2 x-pool WAR dependency.
  - Per tile: t_ first (tiny gate), then fi1 k-outer (4 matmuls per
    arriving chunk), then ct1/ci1, then the PREVIOUS tile's folded output
    layers (one-tile software pipeline keeps wV/wT deadlines late).
  - Output stores are f16 on the sync queue; the kernel's closing block
    computes in two half-column PSUM groups with acts split over
    scalar+vector and stores over both HWDGE rings, so the final
    store chain gates on 256 columns instead of 512.
"""

import numpy as np

B, IMG, TAB, HID = 32768, 2048, 128, 512
NCORES = 8
BS = B // NCORES  # rows per core
NT = 512          # batch-tile (matmul moving/free dim)
KI = IMG // 128   # 16 contraction chunks for fi1
XALL = KI + 1     # + the t chunk, packed as chunk 0 of the same tile
NWARM = 0         # PE p-state warm-up matmuls (on the wt1 tile, earliest load)

_CACHE = {}


def _pack_blocks(WT: np.ndarray, K: int, M: int) -> np.ndarray:
    """[K*128, M*128] -> [128, K, M*128] with [p, k, m*128+j] = WT[k*128+p, m*128+j]."""
    out = WT.reshape(K, 128, M * 128).transpose(1, 0, 2)
    return np.ascontiguousarray(out, dtype=np.float16)


def _build_nc(bs: int):
    import concourse.bass as bass
    import concourse.tile as tile
    from concourse import bacc, mybir

    f32 = mybir.dt.float32
    f16 = mybir.dt.float16
    Relu = mybir.ActivationFunctionType.Relu
    Ident = mybir.ActivationFunctionType.Identity
    ntiles = bs // NT

    nc = bacc.Bacc("TRN2", target_bir_lowering=False, debug=False)

    # tile-major input layout: per batch-tile, all 17 chunks contiguous per
    # partition (16KB lines -> large DMA descriptors, one prefetch per tile)
    iT_d = nc.dram_tensor("iT", [bs // NT, 128, XALL, NT], f16,
                          kind="ExternalInput").ap()
    w_fi1_d = nc.dram_tensor("w_fi1", [128, KI, 512], f16, kind="ExternalInput").ap()
    w_ft1_d = nc.dram_tensor("w_ft1", [128, 1, 512], f16, kind="ExternalInput").ap()
    w_ci1_d = nc.dram_tensor("w_ci1", [128, 4, 512], f16, kind="ExternalInput").ap()
    w_ct1_d = nc.dram_tensor("w_ct1", [128, 4, 512], f16, kind="ExternalInput").ap()
    w_V_d = nc.dram_tensor("w_V", [128, 8, 512], f16, kind="ExternalInput").ap()
    w_T_d = nc.dram_tensor("w_T", [128, 8, 512], f16, kind="ExternalInput").ap()
    bias_d = nc.dram_tensor("bias", [128, 24], f32, kind="ExternalInput").ap()
    # f16 output: halves store traffic and the closing transfer; adds only
    # ~1e-4 to the (2e-2-gated) rel err -- host upcasts to f32.
    out_d = nc.dram_tensor("outT", [2 * HID, bs], f16, kind="ExternalOutput").ap()

    # fi1 chunk groups for tile 0 (i-chunk indices): group completion is
    # all-or-nothing, so the FIRST groups are single chunks (earliest
    # availability while the PE is still ramping); the back groups are
    # coarse since they arrive far ahead of consumption anyway.  x tile
    # chunk index = i-chunk + 1 (chunk 0 is the t chunk).
    WGRP = [(0, 1), (1, 2), (2, 4), (4, 6), (6, 9), (9, 12), (12, 16)]

    with tile.TileContext(nc) as tc:
        with (
            tc.tile_pool(name="w", bufs=1) as wpool,
            tc.tile_pool(name="x", bufs=2) as xpool,
            tc.tile_pool(name="h", bufs=8) as hpool,
            tc.tile_pool(name="o", bufs=8) as opool,
            tc.tile_pool(name="ps", bufs=8, space="PSUM") as pspool,
        ):
            wf1 = wpool.tile([128, KI, 512], f16, name="w_fi1_t")
            wt1 = wpool.tile([128, 1, 512], f16, name="w_ft1_t")
            wc1 = wpool.tile([128, 4, 512], f16, name="w_ci1_t")
            wc2 = wpool.tile([128, 4, 512], f16, name="w_ct1_t")
            wV = wpool.tile([128, 8, 512], f16, name="w_V_t")
            wT = wpool.tile([128, 8, 512], f16, name="w_T_t")
            bt = wpool.tile([128, 24], f32, name="bias_t")

            # ---- preamble loads: ONLY what tile 0's t_/i_ layers need.
            # 8 cores flood the shared HBM at startup, so every byte issued
            # here delays the startup-critical chunks on every core.  All
            # later-needed tensors (wc, wV/wT, x tile 1) are issued from the
            # scalar queue BEHIND act instructions inside the n=0 body: the
            # scalar queue is FIFO and acts wait on PE sems, so those
            # transfers only start once tile 0 is well underway.
            # wt1 and x0's t-chunk gate the first real layer (t_): one on
            # each HWDGE ring so their ~2.7us startup transfers overlap.
            x_cur = xpool.tile([128, XALL, NT], f16, tag="x", name="x_0")
            x_1 = xpool.tile([128, XALL, NT], f16, tag="x", name="x_1")
            nc.sync.dma_start(wt1[:], w_ft1_d[:])
            nc.scalar.dma_start(x_cur[:, 0, :], iT_d[0, :, 0, :])
            for gi, (a, b) in enumerate(WGRP):
                nc.sync.dma_start(x_cur[:, a + 1:b + 1, :], iT_d[0, :, a + 1:b + 1, :])
                nc.scalar.dma_start(wf1[:, a:b, :], w_fi1_d[:, a:b, :])
                if gi == 0:
                    # bias is only needed by the t_ act (~12us); k0's weight
                    # group outranks it on the scalar ring
                    nc.scalar.dma_start(bt[:], bias_d[:])

            # ---- PE warm-up on wt1 (N=128: cheap): starts the HAM busy
            # window early and bridges the t-chunk/bias DMA latency ----
            if NWARM:
                wps = pspool.tile([128, NT], f32, tag="ps", name="warm_ps")
                for _ in range(NWARM):
                    nc.tensor.matmul(wps[:, 0:128], wt1[:, 0, 0:128],
                                     wt1[:, 0, 0:128], start=True, stop=True)

            def act(ps, htag, n, m, bcol, func):
                h = hpool.tile([128, NT], f16, tag=htag, name=f"{htag}_{n}_{m}")
                nc.scalar.activation(h[:], ps[:], func, bias=bt[:, bcol + m:bcol + m + 1])
                return h

            def layer_k_outer(wt, xs, htag, n, bcol, K):
                """All 4 output blocks accumulate in parallel, k outer: 4
                matmuls per input chunk k (rate-matches chunked DMA arrival)."""
                ps = [pspool.tile([128, NT], f32, tag="ps", name=f"ps_{htag}_{n}_{m}")
                      for m in range(4)]
                for k in range(K):
                    for m in range(4):
                        nc.tensor.matmul(ps[m][:], wt[:, k, m * 128:(m + 1) * 128],
                                         xs[k], start=(k == 0), stop=(k == K - 1))
                return [act(ps[m], htag, n, m, bcol, Relu) for m in range(4)]

            def layer_m_outer(wt, xs, htag, n, bcol, K):
                """m outer: each PSUM bank closes after its k loop and drains
                on the scalar engine while the PE works on the next block."""
                outs = []
                for m in range(4):
                    ps = pspool.tile([128, NT], f32, tag="ps", name=f"ps_{htag}_{n}_{m}")
                    for k in range(K):
                        nc.tensor.matmul(ps[:], wt[:, k, m * 128:(m + 1) * 128],
                                         xs[k], start=(k == 0), stop=(k == K - 1))
                    outs.append(act(ps, htag, n, m, bcol, Relu))
                return outs

            def cat_layer(wt, ts_a, ts_b, n, bcol, oname, orow0, final=False):
                """out[m] = sum_k w[k].T@xs_a[k] + w[4+k].T@xs_b[k] + bias; f16 store.

                The bias-add alternates between the scalar and (otherwise idle)
                vector engines so the final tile's four output blocks drain in
                parallel; stores issue from the sync queue, idle by then. The
                very last store (final, m=3) goes in two column halves on two
                queues so the closing transfer is half as long.
                """
                xs_a = [t[:] for t in ts_a]
                xs_b = [t[:] for t in ts_b]
                for m in range(4):
                    o = opool.tile([128, NT], f16, tag="o", name=f"o{oname}_{n}_{m}")
                    bias_ap = bt[:, bcol + m:bcol + m + 1]
                    rows = out_d[orow0 + 128 * m:orow0 + 128 * (m + 1),
                                 n * NT:(n + 1) * NT]
                    if final and m == 3:
                        # kernel's closing chain: two half-column PSUM groups,
                        # so half 0's act+store issue while half 1's matmuls
                        # still run, and the final act gates only 256 cols.
                        # Acts split over scalar+vector; stores over both
                        # HWDGE rings.
                        H = NT // 2
                        for h, (eng_act, eng_dma) in enumerate(
                                ((nc.scalar, nc.sync), (nc.vector, nc.scalar))):
                            ph = pspool.tile([128, H], f32, tag="ps",
                                             name=f"ps_{oname}_{n}_3{h}")
                            cs = slice(h * H, h * H + H)
                            for k in range(4):
                                nc.tensor.matmul(
                                    ph[:], wt[:, k, m * 128:(m + 1) * 128],
                                    ts_a[k][:, cs], start=(k == 0), stop=False)
                            for k in range(4):
                                nc.tensor.matmul(
                                    ph[:], wt[:, 4 + k, m * 128:(m + 1) * 128],
                                    ts_b[k][:, cs], start=False, stop=(k == 3))
                            if eng_act is nc.scalar:
                                nc.scalar.activation(o[:, cs], ph[:], Ident,
                                                     bias=bias_ap)
                            else:
                                nc.vector.tensor_scalar_add(o[:, cs], ph[:],
                                                            bias_ap)
                            eng_dma.dma_start(rows[:, cs], o[:, cs])
                        continue
                    ps = pspool.tile([128, NT], f32, tag="ps", name=f"ps_{oname}_{n}_{m}")
                    for k in range(4):
                        nc.tensor.matmul(ps[:], wt[:, k, m * 128:(m + 1) * 128],
                                         xs_a[k], start=(k == 0), stop=False)
                    for k in range(4):
                        nc.tensor.matmul(ps[:], wt[:, 4 + k, m * 128:(m + 1) * 128],
                                         xs_b[k], start=False, stop=(k == 3))
                    if m % 2 == 0:
                        nc.scalar.activation(o[:], ps[:], Ident, bias=bias_ap)
                    else:
                        nc.vector.tensor_scalar_add(o[:], ps[:], bias_ap)
                    nc.sync.dma_start(rows, o[:])

            def vt_phase(n, v1, i_, v2, t_, final=False):
                # ---- V = [v1, i_] @ WcatV.T + bcatV ----
                cat_layer(wV, v1, i_, n, 16, "V", 0)
                # ---- T = [v2, t_] @ WcatT.T + bcatT ----
                cat_layer(wT, v2, t_, n, 20, "T", HID, final=final)

            xtiles = [x_cur, x_1]
            prev = None
            for n in range(ntiles):
                x_n = xtiles[n]
                xs_i = [x_n[:, k + 1, :] for k in range(KI)]

                # JIT prefetch of tile n+1 (x pool bufs=2: the issue WARs the
                # tile n-1 slot, so the transfer starts right as tile n does
                # -- a full tile-time (~35us) ahead of need, and never during
                # the 8-core startup HBM crunch)
                if n >= 1 and n + 1 < ntiles:
                    x_nxt = xpool.tile([128, XALL, NT], f16, tag="x", name=f"x_{n + 1}")
                    nc.sync.dma_start(x_nxt[:, 0:9, :], iT_d[n + 1, :, 0:9, :])
                    nc.sync.dma_start(x_nxt[:, 9:XALL, :], iT_d[n + 1, :, 9:XALL, :])
                    xtiles.append(x_nxt)

                # ---- t_ = relu(t @ ft1.T + b): gates on only 256KB of input ----
                t_ = layer_m_outer(wt1, [x_n[:, 0, :]], "t_", n, 4, 1)
                if n == 0:
                    # wc loads ride the scalar queue behind the t_ acts:
                    # transfer starts once tile 0 is underway, lands well
                    # before the v2/v1 layers need them
                    nc.scalar.dma_start(wc2[:], w_ct1_d[:])
                    nc.scalar.dma_start(wc1[:], w_ci1_d[:])
                # ---- i_ = relu(i @ fi1.T + b) ----
                i_ = layer_k_outer(wf1, xs_i, "i_", n, 0, KI)
                if n == 0:
                    # x tile 1 behind the i_ acts (needed from ~tile 1 mid)
                    nc.scalar.dma_start(x_1[:, 0:9, :], iT_d[1, :, 0:9, :])
                    nc.scalar.dma_start(x_1[:, 9:XALL, :], iT_d[1, :, 9:XALL, :])

                # ---- v2 = relu(t_ @ ct1.T + b): fills the i_ activation latency ----
                v2 = layer_m_outer(wc2, [h[:] for h in t_], "v2", n, 12, 4)
                # ---- v1 = relu(i_ @ ci1.T + b) ----
                v1 = layer_m_outer(wc1, [h[:] for h in i_], "v1", n, 8, 4)
                if n == 0:
                    # cat weights behind the v2/v1 acts (needed from the
                    # deferred vt_phase(0), which runs inside tile 1)
                    nc.scalar.dma_start(wV[:, 0:4, :], w_V_d[:, 0:4, :])
                    nc.scalar.dma_start(wV[:, 4:8, :], w_V_d[:, 4:8, :])
                    nc.scalar.dma_start(wT[:, 0:4, :], w_T_d[:, 0:4, :])
                    nc.scalar.dma_start(wT[:, 4:8, :], w_T_d[:, 4:8, :])

                # ---- V/T of the PREVIOUS tile (one-tile software pipeline:
                # moves the wV/wT load deadlines out of the startup crunch) ----
                if prev is not None:
                    vt_phase(n - 1, *prev)
                prev = (v1, i_, v2, t_)

            vt_phase(ntiles - 1, *prev, final=True)

    nc.compile()
    return nc


def _host_pack(inp: dict):
    f8 = np.float64
    fi1_w, fi1_b = inp["fi1_w"], inp["fi1_b"]
    ft1_w, ft1_b = inp["ft1_w"], inp["ft1_b"]
    ci1_w, ci1_b = inp["ci1_w"], inp["ci1_b"]
    ct1_w, ct1_b = inp["ct1_w"], inp["ct1_b"]

    def fold(wv, bv, wo, bo, f_w, f_b):
        Wvo = wo.astype(f8) @ wv.astype(f8)
        bvo = wo.astype(f8) @ bv.astype(f8) + bo.astype(f8)
        Wcat = np.concatenate([f_w.astype(f8) @ Wvo, f_w.astype(f8)], axis=1)
        bcat = f_w.astype(f8) @ bvo + f_b.astype(f8)
        return Wcat.astype(np.float32), bcat.astype(np.float32)

    WcatV, bcatV = fold(inp["aV_wv"], inp["aV_bv"], inp["aV_wo"], inp["aV_bo"],
                        inp["fi2_w"], inp["fi2_b"])
    WcatT, bcatT = fold(inp["aT_wv"], inp["aT_bv"], inp["aT_wo"], inp["aT_bo"],
                        inp["ft2_w"], inp["ft2_b"])

    weights = {
        "w_fi1": _pack_blocks(np.ascontiguousarray(fi1_w.T), 16, 4),
        "w_ft1": _pack_blocks(np.ascontiguousarray(ft1_w.T), 1, 4),
        "w_ci1": _pack_blocks(np.ascontiguousarray(ci1_w.T), 4, 4),
        "w_ct1": _pack_blocks(np.ascontiguousarray(ct1_w.T), 4, 4),
        "w_V": _pack_blocks(np.ascontiguousarray(WcatV.T), 8, 4),
        "w_T": _pack_blocks(np.ascontiguousarray(WcatT.T), 8, 4),
    }
    cols = []
    for b in (fi1_b, ft1_b, ci1_b, ct1_b, bcatV, bcatT):
        for m in range(4):
            cols.append(b[128 * m:128 * (m + 1)])
    weights["bias"] = np.ascontiguousarray(np.stack(cols, axis=1), dtype=np.float32)
    return weights


def make_in_maps(inputs: dict):
    """Full inputs -> per-core input dicts (shard batch, replicate weights)."""
    inputs = {k: np.asarray(v) for k, v in inputs.items()}
    i = np.asarray(inputs["i"], dtype=np.float32)
    t = np.asarray(inputs["t"], dtype=np.float32)
    weights = _host_pack(inputs)
    i16 = i.astype(np.float16)
    t16 = t.astype(np.float16)
    ntiles = BS // NT
    in_maps = []
    for c in range(NCORES):
        sl = slice(c * BS, (c + 1) * BS)
        m = dict(weights)
        # [ntiles, 128, XALL, NT]: batch-tile major; chunk 0 holds
        # t[n*NT+j, p], chunk 1+k holds i[n*NT+j, 128k+p].
        xi = i16[sl].T.reshape(KI, 128, ntiles, NT)   # [k, p, n, j]
        xt = t16[sl].T.reshape(TAB, ntiles, NT)       # [p, n, j]
        full = np.empty((ntiles, 128, XALL, NT), dtype=np.float16)
        full[:, :, 0, :] = xt.transpose(1, 0, 2)
        full[:, :, 1:, :] = xi.transpose(2, 1, 0, 3)
        m["iT"] = full
        in_maps.append(m)
    return in_maps


def kernel(**inputs) -> np.ndarray:
    from concourse import bass_utils

    if "nc" not in _CACHE:
        _CACHE["nc"] = _build_nc(BS)
    nc = _CACHE["nc"]

    in_maps = make_in_maps(inputs)
    res = bass_utils.run_bass_kernel_spmd(nc, in_maps, core_ids=list(range(NCORES)))

    out = np.empty((B, 2 * HID), dtype=np.float32)
    for c in range(NCORES):
        out[c * BS:(c + 1) * BS] = res.results[c]["outT"].T.astype(np.float32)
    return out

